# revision 9
# baseline (speedup 1.0000x reference)
import sys
sys.path.insert(0, "/opt/trn_rl_repo")
import numpy as np

N_NODES = 100000
F_DIM = 256
H = 128
BATCH = 4096
N_USERS = 50000
NCORES = 8
SHARD = N_NODES // NCORES          # 12500
SHARD_PAD = 12544                  # 98*128
NTILES = SHARD_PAD // 128          # 98
LN_EPS = 1e-5

NPASS = 4
BUCKET = 25000
# fixed wave sizes (tokens, multiples of 128); wave k = k-th edge of each dest row
W_FIXED = [11264, 7808, 4352, 1920, 768, 384] + [128] * 10
TOK_P = sum(W_FIXED)               # tokens per (pass) = 27776
W_OFF = np.cumsum([0] + W_FIXED)[:-1]
S_ROWS = 12928                     # 101*128 ; rows >= TRASH are a trash bin
TRASH = 12800
BMAX = 896                         # routed score slots per core (mean 512, +18 sigma)

_compiled = None


def _wrap16(idx):
    n = len(idx)
    a = np.zeros((128, n // 16), np.int16)
    p = np.arange(n)
    for r in range(0, 128, 16):
        a[r + p % 16, p // 16] = idx
    return a


def _tokwrap(v):
    n = len(v)
    a = np.zeros((128, n // 128), v.dtype)
    p = np.arange(n)
    a[p % 128, p // 128] = v
    return a


def _prep_spmm(c, adj_row, adj_col, adj_vals):
    """Per-core wave structures: gidx/sidx [NPASS,128,TOK_P//16] i16, vals [NPASS,128,TOK_P//128] f32."""
    base = c * SHARD
    m = (adj_row >= base) & (adj_row < base + SHARD)
    er = (adj_row[m] - base).astype(np.int64)
    ec = adj_col[m].astype(np.int64)
    ev = adj_vals[m].astype(np.float32)

    gidx = np.zeros((NPASS, 128, TOK_P // 16), np.int16)
    sidx = np.zeros((NPASS, 128, TOK_P // 16), np.int16)
    vals = np.zeros((NPASS, 128, TOK_P // 128), np.float32)

    for b in range(NPASS):
        pm = (ec >= b * BUCKET) & (ec < (b + 1) * BUCKET)
        pr, pc, pv = er[pm], (ec[pm] - b * BUCKET), ev[pm]
        order = np.argsort(pr, kind="stable")
        pr, pc, pv = pr[order], pc[order], pv[order]
        rows, starts = np.unique(pr, return_index=True)
        degs = np.append(starts[1:], len(pr)) - starts
        if degs.size and degs.max() > len(W_FIXED):
            raise RuntimeError(f"degree {degs.max()} exceeds {len(W_FIXED)} waves")
        gp = np.zeros(TOK_P, np.int64)
        sp = np.full(TOK_P, TRASH, np.int64)
        vp = np.zeros(TOK_P, np.float32)
        for k, wk in enumerate(W_FIXED):
            sel = degs > k
            nsel = int(sel.sum())
            if nsel == 0:
                break
            if nsel > wk:
                raise RuntimeError(f"wave {k} overflow: {nsel} > {wk}")
            o = W_OFF[k]
            gp[o : o + nsel] = pc[starts[sel] + k]
            sp[o : o + nsel] = rows[sel]
            vp[o : o + nsel] = pv[starts[sel] + k]
        gidx[b] = _wrap16(gp)
        sidx[b] = _wrap16(sp)
        vals[b] = _tokwrap(vp)
    return gidx, sidx, vals


def _build():
    import os
    STAGE = int(os.environ.get("K_STAGE", "5"))
    from concourse import bass, bacc, tile
    import concourse.mybir as mybir

    f32 = mybir.dt.float32
    i16 = mybir.dt.int16
    nc = bacc.Bacc("TRN2", target_bir_lowering=False, debug=False, num_devices=NCORES)

    # ---- inputs (per core) ----
    nfT_d = nc.dram_tensor("nfT", [F_DIM, SHARD_PAD], f32, kind="ExternalInput")
    wpT_d = nc.dram_tensor("wpT", [F_DIM, H], f32, kind="ExternalInput")
    w1T_d = nc.dram_tensor("w1T", [H, H], f32, kind="ExternalInput")
    w2T_d = nc.dram_tensor("w2T", [H, H], f32, kind="ExternalInput")
    bc_d = nc.dram_tensor("bcasts", [128, 5, H], f32, kind="ExternalInput")  # bproj,b1,b2,g,lnb
    iid_d = nc.dram_tensor("iid", [SHARD_PAD, H], f32, kind="ExternalInput")
    ident_d = nc.dram_tensor("ident", [128, 128], f32, kind="ExternalInput")
    gidx_d = nc.dram_tensor("gidx", [NPASS, 128, TOK_P // 16], i16, kind="ExternalInput")
    sidx_d = nc.dram_tensor("sidx", [NPASS, 128, TOK_P // 16], i16, kind="ExternalInput")
    vals_d = nc.dram_tensor("vals", [NPASS, 128, TOK_P // 128], f32, kind="ExternalInput")
    iidx_d = nc.dram_tensor("iidx", [128, BMAX // 16], i16, kind="ExternalInput")
    urows_d = nc.dram_tensor("urows", [BMAX, H], f32, kind="ExternalInput")
    bsum_d = nc.dram_tensor("bsum", [128, BMAX // 128], f32, kind="ExternalInput")

    scores_d = nc.dram_tensor("scores", [128, BMAX // 128], f32, kind="ExternalOutput")

    # ---- internal DRAM ----
    h0_d = nc.dram_tensor("h0loc", [SHARD_PAD, H], f32)       # proj out (residual)
    h1_d = nc.dram_tensor("h1loc", [SHARD_PAD, H], f32)
    item_d = nc.dram_tensor("itemloc", [S_ROWS, H], f32)
    s1_d = nc.dram_tensor("s1", [S_ROWS, H], f32)
    s2_d = nc.dram_tensor("s2", [S_ROWS, H], f32)
    H0_d = nc.dram_tensor("H0", [N_NODES, H], f32, addr_space="Shared")
    H1_d = nc.dram_tensor("H1", [N_NODES, H], f32, addr_space="Shared")

    RG = [list(range(NCORES))]

    with tile.TileContext(nc) as tc:
        with (
            tc.tile_pool(name="const", bufs=1) as constp,
            tc.tile_pool(name="xp", bufs=2) as xp,
            tc.tile_pool(name="idxp", bufs=3) as idxp,
            tc.tile_pool(name="dn", bufs=3) as dnp,
            tc.tile_pool(name="sc", bufs=4) as scp,
            tc.tile_pool(name="ps", bufs=2, space="PSUM") as psp,
        ):
            # constants
            wpT_t = constp.tile([128, 2, H], f32)
            nc.sync.dma_start(wpT_t[:], wpT_d.ap().rearrange("(a p) h -> p a h", p=128))
            w1T_t = constp.tile([H, H], f32)
            nc.sync.dma_start(w1T_t[:], w1T_d[:])
            w2T_t = constp.tile([H, H], f32)
            nc.sync.dma_start(w2T_t[:], w2T_d[:])
            bc_t = constp.tile([128, 5, H], f32)
            nc.sync.dma_start(bc_t[:], bc_d[:])
            ident_t = constp.tile([128, 128], f32)
            nc.sync.dma_start(ident_t[:], ident_d[:])
            zero_t = constp.tile([128, 16, H], f32)
            nc.vector.memset(zero_t[:], 0.0)
            eps_t = constp.tile([128, 1], f32)
            nc.vector.memset(eps_t[:], LN_EPS)

            BPROJ, B1, B2, LNG, LNB = range(5)

            # ---------- proj: h0 = nf @ WpT + bproj ----------
            for t in range(NTILES):
                lhs_a = dnp.tile([128, 128], f32, tag="plhs")
                lhs_b = dnp.tile([128, 128], f32, tag="plhs")
                nc.sync.dma_start(lhs_a[:], nfT_d[0:128, t * 128 : (t + 1) * 128])
                nc.sync.dma_start(lhs_b[:], nfT_d[128:256, t * 128 : (t + 1) * 128])
                ps = psp.tile([128, H], f32, tag="pps")
                nc.tensor.matmul(ps[:], lhs_a[:], wpT_t[:, 0, :], start=True, stop=False)
                nc.tensor.matmul(ps[:], lhs_b[:], wpT_t[:, 1, :], start=False, stop=True)
                h0_t = dnp.tile([128, H], f32, tag="ph0")
                nc.vector.tensor_tensor(h0_t[:], ps[:], bc_t[:, BPROJ, :], mybir.AluOpType.add)
                nc.sync.dma_start(h0_d[t * 128 : (t + 1) * 128, :], h0_t[:])

            # exchange h0 -> H0 (full)
            nc.gpsimd.collective_compute(
                "AllGather", mybir.AluOpType.bypass, replica_groups=RG,
                ins=[h0_d[0:SHARD, :]], outs=[H0_d[:]],
            )

            def spmm_round(Hfull, s_dram):
                # zero s
                for z in range(0, S_ROWS // 128, 16):
                    zn = min(16, S_ROWS // 128 - z)
                    nc.sync.dma_start(
                        s_dram.ap().rearrange("(a p) h -> p a h", p=128)[:, z : z + zn, :],
                        zero_t[:, 0:zn, :],
                    )
                for b in range(NPASS):
                    for k, wk in enumerate(W_FIXED):
                        cols = wk // 128
                        o16 = W_OFF[k] // 16
                        o128 = W_OFF[k] // 128
                        g_t = idxp.tile([128, wk // 16], i16, tag="gix")
                        s_t = idxp.tile([128, wk // 16], i16, tag="six")
                        v_t = idxp.tile([128, cols], f32, tag="val")
                        nc.sync.dma_start(g_t[:], gidx_d[b, :, o16 : o16 + wk // 16])
                        nc.sync.dma_start(s_t[:], sidx_d[b, :, o16 : o16 + wk // 16])
                        nc.sync.dma_start(v_t[:], vals_d[b, :, o128 : o128 + cols])
                        x_t = xp.tile([128, 88, H], f32, tag="xt")
                        # SWDGE ops limited to 1024 tokens (64 ring entries)
                        for co in range(0, cols, 8):
                            cn = min(8, cols - co)
                            nc.gpsimd.dma_gather(
                                x_t[:, co : co + cn, :],
                                Hfull[b * BUCKET : (b + 1) * BUCKET, :],
                                g_t[:, co * 8 : (co + cn) * 8], cn * 128, cn * 128, H,
                            )
                        # scale by vals: alternate DVE / ACT per column
                        for cc in range(cols):
                            if cc % 2 == 0:
                                nc.vector.tensor_scalar(
                                    x_t[:, cc, :], x_t[:, cc, :], v_t[:, cc : cc + 1],
                                    None, mybir.AluOpType.mult)
                            else:
                                nc.scalar.mul(x_t[:, cc, :], x_t[:, cc, :], v_t[:, cc : cc + 1])
                        for co in range(0, cols, 8):
                            cn = min(8, cols - co)
                            nc.gpsimd.dma_scatter_add(
                                s_dram[:], x_t[:, co : co + cn, :],
                                s_t[:, co * 8 : (co + cn) * 8], cn * 128, cn * 128, H)

            def dense_round(s_dram, wT_t, bias_i, out_cb):
                for t in range(NTILES):
                    st = dnp.tile([128, H], f32, tag="dst")
                    nc.sync.dma_start(st[:], s_dram[t * 128 : (t + 1) * 128, :])
                    psT = psp.tile([128, 128], f32, tag="dpsT")
                    nc.tensor.transpose(psT[:], st[:], ident_t[:])
                    sT = dnp.tile([128, 128], f32, tag="dsT")
                    nc.vector.tensor_copy(sT[:], psT[:])
                    ps = psp.tile([128, H], f32, tag="dps")
                    nc.tensor.matmul(ps[:], sT[:], wT_t[:], start=True, stop=True)
                    zb = dnp.tile([128, H], f32, tag="dzb")
                    nc.vector.tensor_tensor(zb[:], ps[:], bc_t[:, bias_i, :], mybir.AluOpType.add)
                    zr = dnp.tile([128, H], f32, tag="dzr")
                    rsum = dnp.tile([128, 1], f32, tag="drs")
                    nc.scalar.activation(zr[:], zb[:], mybir.ActivationFunctionType.Relu,
                                         accum_out=rsum[:])
                    mean = dnp.tile([128, 1], f32, tag="dmn")
                    nc.scalar.mul(mean[:], rsum[:], 1.0 / H)
                    zc = dnp.tile([128, H], f32, tag="dzc")
                    nc.vector.tensor_scalar(zc[:], zr[:], mean[:], None,
                                            mybir.AluOpType.subtract)
                    sq = dnp.tile([128, H], f32, tag="dsq")
                    ssq = dnp.tile([128, 1], f32, tag="dsq1")
                    nc.scalar.activation(sq[:], zc[:], mybir.ActivationFunctionType.Square,
                                         accum_out=ssq[:])
                    std = dnp.tile([128, 1], f32, tag="dsd")
                    nc.scalar.activation(std[:], ssq[:], mybir.ActivationFunctionType.Sqrt,
                                         bias=eps_t[:], scale=1.0 / H)
                    inv = dnp.tile([128, 1], f32, tag="din")
                    nc.vector.reciprocal(inv[:], std[:])
                    t1 = dnp.tile([128, H], f32, tag="dt1")
                    nc.vector.scalar_tensor_tensor(t1[:], zc[:], inv[:], bc_t[:, LNG, :],
                                                   mybir.AluOpType.mult, mybir.AluOpType.mult)
                    hout = dnp.tile([128, H], f32, tag="dho")
                    nc.vector.tensor_tensor(hout[:], t1[:], bc_t[:, LNB, :], mybir.AluOpType.add)
                    out_cb(t, hout)

            # ---------- round 1 ----------
            if STAGE >= 2:
                spmm_round(H0_d, s1_d)

            def r1_out(t, hout):
                nc.sync.dma_start(h1_d[t * 128 : (t + 1) * 128, :], hout[:])

            if STAGE >= 3:
                dense_round(s1_d, w1T_t, B1, r1_out)
                nc.gpsimd.collective_compute(
                    "AllGather", mybir.AluOpType.bypass, replica_groups=RG,
                    ins=[h1_d[0:SHARD, :]], outs=[H1_d[:]],
                )

            # ---------- round 2 + residual + item_id_emb ----------
            if STAGE >= 4:
                spmm_round(H1_d, s2_d)

            def r2_out(t, hout):
                r0 = dnp.tile([128, H], f32, tag="dr0")
                nc.sync.dma_start(r0[:], h0_d[t * 128 : (t + 1) * 128, :])
                ii = dnp.tile([128, H], f32, tag="dii")
                nc.sync.dma_start(ii[:], iid_d[t * 128 : (t + 1) * 128, :])
                e1 = dnp.tile([128, H], f32, tag="de1")
                nc.vector.tensor_tensor(e1[:], hout[:], r0[:], mybir.AluOpType.add)
                e2 = dnp.tile([128, H], f32, tag="de2")
                nc.vector.tensor_tensor(e2[:], e1[:], ii[:], mybir.AluOpType.add)
                nc.sync.dma_start(item_d[t * 128 : (t + 1) * 128, :], e2[:])

            if STAGE >= 4:
                dense_round(s2_d, w2T_t, B2, r2_out)

            # ---------- scoring ----------
            bs_t = scp.tile([128, BMAX // 128], f32)
            nc.sync.dma_start(bs_t[:], bsum_d[:])
            if STAGE >= 5:
                iix_t = scp.tile([128, BMAX // 16], i16)
                nc.sync.dma_start(iix_t[:], iidx_d[:])
                it_t = scp.tile([128, BMAX // 128, H], f32)
                nc.gpsimd.dma_gather(it_t[:], item_d[:], iix_t[:], BMAX, BMAX, H)
                u_t = scp.tile([128, BMAX // 128, H], f32)
                nc.sync.dma_start(u_t[:], urows_d.ap().rearrange("(c p) h -> p c h", p=128))
                pr_t = scp.tile([128, BMAX // 128, H], f32)
                nc.vector.tensor_tensor(pr_t[:], it_t[:], u_t[:], mybir.AluOpType.mult)
                dot_t = scp.tile([128, BMAX // 128], f32)
                nc.vector.tensor_reduce(dot_t[:], pr_t[:], mybir.AxisListType.X,
                                        mybir.AluOpType.add)
                sco_t = scp.tile([128, BMAX // 128], f32)
                nc.vector.tensor_tensor(sco_t[:], dot_t[:], bs_t[:], mybir.AluOpType.add)
                nc.sync.dma_start(scores_d[:], sco_t[:])
            else:
                nc.sync.dma_start(scores_d[:], bs_t[:])

    nc.compile()
    return nc


def kernel(**inputs):
    global _compiled
    nf = np.asarray(inputs["node_features"], np.float32)
    adj_row = np.asarray(inputs["adj_row"], np.int64)
    adj_col = np.asarray(inputs["adj_col"], np.int64)
    adj_vals = np.asarray(inputs["adj_vals"], np.float32)
    user_idx = np.asarray(inputs["user_idx"], np.int64)
    item_idx = np.asarray(inputs["item_idx"], np.int64)
    W_proj = np.asarray(inputs["W_proj"], np.float32)
    b_proj = np.asarray(inputs["b_proj"], np.float32)
    W1 = np.asarray(inputs["W1"], np.float32)
    b1 = np.asarray(inputs["b1"], np.float32)
    W2 = np.asarray(inputs["W2"], np.float32)
    b2 = np.asarray(inputs["b2"], np.float32)
    ln_g = np.asarray(inputs["ln_g"], np.float32)
    ln_b = np.asarray(inputs["ln_b"], np.float32)
    user_emb = np.asarray(inputs["user_emb"], np.float32)
    item_id_emb = np.asarray(inputs["item_id_emb"], np.float32)
    user_bias = np.asarray(inputs["user_bias"], np.float32)
    item_bias = np.asarray(inputs["item_bias"], np.float32)
    global_bias = np.asarray(inputs["global_bias"], np.float32)

    if _compiled is None:
        _compiled = _build()
    nc = _compiled

    bcasts = np.ascontiguousarray(np.stack([
        np.broadcast_to(b_proj, (128, H)),
        np.broadcast_to(b1, (128, H)),
        np.broadcast_to(b2, (128, H)),
        np.broadcast_to(ln_g, (128, H)),
        np.broadcast_to(ln_b, (128, H)),
    ]).transpose(1, 0, 2)).astype(np.float32)
    ident = np.eye(128, dtype=np.float32)

    in_maps = []
    routing = []
    core_of = item_idx // SHARD
    for c in range(NCORES):
        gidx, sidx, vals = _prep_spmm(c, adj_row, adj_col, adj_vals)
        nfT = np.zeros((F_DIM, SHARD_PAD), np.float32)
        nfT[:, :SHARD] = nf[c * SHARD : (c + 1) * SHARD].T
        iid = np.zeros((SHARD_PAD, H), np.float32)
        iid[:SHARD] = item_id_emb[c * SHARD : (c + 1) * SHARD]

        pos = np.nonzero(core_of == c)[0]
        if len(pos) > BMAX:
            raise RuntimeError(f"core {c}: {len(pos)} routed items > {BMAX}")
        il = np.zeros(BMAX, np.int64)
        il[: len(pos)] = item_idx[pos] - c * SHARD
        ur = np.zeros((BMAX, H), np.float32)
        ur[: len(pos)] = user_emb[user_idx[pos]]
        bs = np.zeros(BMAX, np.float32)
        bs[: len(pos)] = (
            user_bias[user_idx[pos], 0] + item_bias[item_idx[pos], 0] + global_bias[0]
        )
        routing.append(pos)
        in_maps.append({
            "nfT": nfT, "wpT": W_proj.T.copy(), "w1T": W1.T.copy(), "w2T": W2.T.copy(),
            "bcasts": bcasts, "iid": iid, "ident": ident,
            "gidx": gidx, "sidx": sidx, "vals": vals,
            "iidx": _wrap16(il), "urows": ur, "bsum": _tokwrap(bs),
        })

    from concourse.bass_utils import run_bass_kernel_spmd
    res = run_bass_kernel_spmd(nc, in_maps, list(range(NCORES)))

    out = np.zeros(BATCH, np.float32)
    for c in range(NCORES):
        sc = res.results[c]["scores"]
        pos = routing[c]
        t = np.arange(len(pos))
        out[pos] = sc[t % 128, t // 128]
    return out


# revision 10
# speedup vs baseline: 1.1685x; 1.1685x over previous
import sys
sys.path.insert(0, "/opt/trn_rl_repo")
import numpy as np

N_NODES = 100000
F_DIM = 256
H = 128
BATCH = 4096
N_USERS = 50000
NCORES = 8
SHARD = N_NODES // NCORES          # 12500
SHARD_PAD = 12544                  # 98*128
NTILES = SHARD_PAD // 128          # 98
LN_EPS = 1e-5

NPASS = 4
BUCKET = 25000
# fixed wave sizes (tokens, multiples of 128); wave k = k-th edge of each dest row
W_FIXED = [11264, 7808, 4352, 1920, 768, 384] + [128] * 10
TOK_P = sum(W_FIXED)               # tokens per (pass) = 27776
W_OFF = np.cumsum([0] + W_FIXED)[:-1]
S_ROWS = 12928                     # 101*128 ; rows >= TRASH are a trash bin
TRASH = 12800
BMAX = 896                         # routed score slots per core (mean 512, +18 sigma)

_compiled = None
last_run_ns = None


def _wrap16(idx):
    n = len(idx)
    a = np.zeros((128, n // 16), np.int16)
    p = np.arange(n)
    for r in range(0, 128, 16):
        a[r + p % 16, p // 16] = idx
    return a


def _tokwrap(v):
    n = len(v)
    a = np.zeros((128, n // 128), v.dtype)
    p = np.arange(n)
    a[p % 128, p // 128] = v
    return a


def _prep_spmm(c, adj_row, adj_col, adj_vals):
    """Per-core wave structures: gidx/sidx [NPASS,128,TOK_P//16] i16, vals [NPASS,128,TOK_P//128] f32."""
    base = c * SHARD
    m = (adj_row >= base) & (adj_row < base + SHARD)
    er = (adj_row[m] - base).astype(np.int64)
    ec = adj_col[m].astype(np.int64)
    ev = adj_vals[m].astype(np.float32)

    gidx = np.zeros((NPASS, 128, TOK_P // 16), np.int16)
    sidx = np.zeros((NPASS, 128, TOK_P // 16), np.int16)
    vals = np.zeros((NPASS, 128, TOK_P // 128), np.float32)

    for b in range(NPASS):
        pm = (ec >= b * BUCKET) & (ec < (b + 1) * BUCKET)
        pr, pc, pv = er[pm], (ec[pm] - b * BUCKET), ev[pm]
        order = np.argsort(pr, kind="stable")
        pr, pc, pv = pr[order], pc[order], pv[order]
        rows, starts = np.unique(pr, return_index=True)
        degs = np.append(starts[1:], len(pr)) - starts
        if degs.size and degs.max() > len(W_FIXED):
            raise RuntimeError(f"degree {degs.max()} exceeds {len(W_FIXED)} waves")
        gp = np.zeros(TOK_P, np.int64)
        sp = np.full(TOK_P, TRASH, np.int64)
        vp = np.zeros(TOK_P, np.float32)
        for k, wk in enumerate(W_FIXED):
            sel = degs > k
            nsel = int(sel.sum())
            if nsel == 0:
                break
            if nsel > wk:
                raise RuntimeError(f"wave {k} overflow: {nsel} > {wk}")
            o = W_OFF[k]
            gp[o : o + nsel] = pc[starts[sel] + k]
            sp[o : o + nsel] = rows[sel]
            vp[o : o + nsel] = pv[starts[sel] + k]
        gidx[b] = _wrap16(gp)
        sidx[b] = _wrap16(sp)
        vals[b] = _tokwrap(vp)
    return gidx, sidx, vals


def _build():
    import os
    STAGE = int(os.environ.get("K_STAGE", "5"))
    from concourse import bass, bacc, tile
    import concourse.mybir as mybir

    f32 = mybir.dt.float32
    i16 = mybir.dt.int16
    nc = bacc.Bacc("TRN2", target_bir_lowering=False, debug=False, num_devices=NCORES)

    # ---- inputs (per core) ----
    nfT_d = nc.dram_tensor("nfT", [F_DIM, SHARD_PAD], f32, kind="ExternalInput")
    wpT_d = nc.dram_tensor("wpT", [F_DIM, H], f32, kind="ExternalInput")
    w1T_d = nc.dram_tensor("w1T", [H, H], f32, kind="ExternalInput")
    w2T_d = nc.dram_tensor("w2T", [H, H], f32, kind="ExternalInput")
    bc_d = nc.dram_tensor("bcasts", [128, 5, H], f32, kind="ExternalInput")  # bproj,b1,b2,g,lnb
    iid_d = nc.dram_tensor("iid", [SHARD_PAD, H], f32, kind="ExternalInput")
    ident_d = nc.dram_tensor("ident", [128, 128], f32, kind="ExternalInput")
    gidx_d = nc.dram_tensor("gidx", [NPASS, 128, TOK_P // 16], i16, kind="ExternalInput")
    sidx_d = nc.dram_tensor("sidx", [NPASS, 128, TOK_P // 16], i16, kind="ExternalInput")
    vals_d = nc.dram_tensor("vals", [NPASS, 128, TOK_P // 128], f32, kind="ExternalInput")
    iidx_d = nc.dram_tensor("iidx", [128, BMAX // 16], i16, kind="ExternalInput")
    urows_d = nc.dram_tensor("urows", [BMAX, H], f32, kind="ExternalInput")
    bsum_d = nc.dram_tensor("bsum", [128, BMAX // 128], f32, kind="ExternalInput")

    scores_d = nc.dram_tensor("scores", [128, BMAX // 128], f32, kind="ExternalOutput")

    # ---- internal DRAM ----
    h0_d = nc.dram_tensor("h0loc", [SHARD_PAD, H], f32)       # proj out (residual)
    h1_d = nc.dram_tensor("h1loc", [SHARD_PAD, H], f32)
    item_d = nc.dram_tensor("itemloc", [S_ROWS, H], f32)
    s1_d = nc.dram_tensor("s1", [S_ROWS, H], f32)
    s2_d = nc.dram_tensor("s2", [S_ROWS, H], f32)
    H0_d = nc.dram_tensor("H0", [N_NODES, H], f32, addr_space="Shared")
    H1_d = nc.dram_tensor("H1", [N_NODES, H], f32, addr_space="Shared")

    RG = [list(range(NCORES))]

    with tile.TileContext(nc) as tc:
        with (
            tc.tile_pool(name="const", bufs=1) as constp,
            tc.tile_pool(name="xp", bufs=2) as xp,
            tc.tile_pool(name="idxp", bufs=3) as idxp,
            tc.tile_pool(name="dn", bufs=3) as dnp,
            tc.tile_pool(name="sc", bufs=4) as scp,
            tc.tile_pool(name="ps", bufs=2, space="PSUM") as psp,
        ):
            # constants
            wpT_t = constp.tile([128, 2, H], f32)
            nc.sync.dma_start(wpT_t[:], wpT_d.ap().rearrange("(a p) h -> p a h", p=128))
            w1T_t = constp.tile([H, H], f32)
            nc.sync.dma_start(w1T_t[:], w1T_d[:])
            w2T_t = constp.tile([H, H], f32)
            nc.sync.dma_start(w2T_t[:], w2T_d[:])
            bc_t = constp.tile([128, 5, H], f32)
            nc.sync.dma_start(bc_t[:], bc_d[:])
            ident_t = constp.tile([128, 128], f32)
            nc.sync.dma_start(ident_t[:], ident_d[:])
            zero_t = constp.tile([128, 16, H], f32)
            nc.vector.memset(zero_t[:], 0.0)
            eps_t = constp.tile([128, 1], f32)
            nc.vector.memset(eps_t[:], LN_EPS)

            BPROJ, B1, B2, LNG, LNB = range(5)

            # ---------- proj: h0 = nf @ WpT + bproj ----------
            for t in range(NTILES):
                lhs_a = dnp.tile([128, 128], f32, tag="plhs")
                lhs_b = dnp.tile([128, 128], f32, tag="plhs")
                nc.sync.dma_start(lhs_a[:], nfT_d[0:128, t * 128 : (t + 1) * 128])
                nc.sync.dma_start(lhs_b[:], nfT_d[128:256, t * 128 : (t + 1) * 128])
                ps = psp.tile([128, H], f32, tag="pps")
                nc.tensor.matmul(ps[:], lhs_a[:], wpT_t[:, 0, :], start=True, stop=False)
                nc.tensor.matmul(ps[:], lhs_b[:], wpT_t[:, 1, :], start=False, stop=True)
                h0_t = dnp.tile([128, H], f32, tag="ph0")
                nc.vector.tensor_tensor(h0_t[:], ps[:], bc_t[:, BPROJ, :], mybir.AluOpType.add)
                nc.sync.dma_start(h0_d[t * 128 : (t + 1) * 128, :], h0_t[:])

            # exchange h0 -> H0 (full)
            nc.gpsimd.collective_compute(
                "AllGather", mybir.AluOpType.bypass, replica_groups=RG,
                ins=[h0_d[0:SHARD, :]], outs=[H0_d[:]],
            )

            def spmm_round(Hfull, s_dram):
                # zero s
                for z in range(0, S_ROWS // 128, 16):
                    zn = min(16, S_ROWS // 128 - z)
                    nc.sync.dma_start(
                        s_dram.ap().rearrange("(a p) h -> p a h", p=128)[:, z : z + zn, :],
                        zero_t[:, 0:zn, :],
                    )
                for b in range(NPASS):
                    for k, wk in enumerate(W_FIXED):
                        cols = wk // 128
                        o16 = W_OFF[k] // 16
                        o128 = W_OFF[k] // 128
                        g_t = idxp.tile([128, wk // 16], i16, tag="gix")
                        s_t = idxp.tile([128, wk // 16], i16, tag="six")
                        v_t = idxp.tile([128, cols], f32, tag="val")
                        nc.sync.dma_start(g_t[:], gidx_d[b, :, o16 : o16 + wk // 16])
                        nc.sync.dma_start(s_t[:], sidx_d[b, :, o16 : o16 + wk // 16])
                        nc.sync.dma_start(v_t[:], vals_d[b, :, o128 : o128 + cols])
                        x_t = xp.tile([128, 88, H], f32, tag="xt")
                        # SWDGE ops limited to 1024 tokens (64 ring entries)
                        for co in range(0, cols, 8):
                            cn = min(8, cols - co)
                            nc.gpsimd.dma_gather(
                                x_t[:, co : co + cn, :],
                                Hfull[b * BUCKET : (b + 1) * BUCKET, :],
                                g_t[:, co * 8 : (co + cn) * 8], cn * 128, cn * 128, H,
                            )
                        # scale by vals: alternate DVE / ACT per column
                        for cc in range(cols):
                            if cc % 2 == 0:
                                nc.vector.tensor_scalar(
                                    x_t[:, cc, :], x_t[:, cc, :], v_t[:, cc : cc + 1],
                                    None, mybir.AluOpType.mult)
                            else:
                                nc.scalar.mul(x_t[:, cc, :], x_t[:, cc, :], v_t[:, cc : cc + 1])
                        for co in range(0, cols, 8):
                            cn = min(8, cols - co)
                            nc.gpsimd.dma_scatter_add(
                                s_dram[:], x_t[:, co : co + cn, :],
                                s_t[:, co * 8 : (co + cn) * 8], cn * 128, cn * 128, H)

            def dense_round(s_dram, wT_t, bias_i, out_cb):
                for t in range(NTILES):
                    st = dnp.tile([128, H], f32, tag="dst")
                    nc.sync.dma_start(st[:], s_dram[t * 128 : (t + 1) * 128, :])
                    psT = psp.tile([128, 128], f32, tag="dpsT")
                    nc.tensor.transpose(psT[:], st[:], ident_t[:])
                    sT = dnp.tile([128, 128], f32, tag="dsT")
                    nc.vector.tensor_copy(sT[:], psT[:])
                    ps = psp.tile([128, H], f32, tag="dps")
                    nc.tensor.matmul(ps[:], sT[:], wT_t[:], start=True, stop=True)
                    zb = dnp.tile([128, H], f32, tag="dzb")
                    nc.vector.tensor_tensor(zb[:], ps[:], bc_t[:, bias_i, :], mybir.AluOpType.add)
                    zr = dnp.tile([128, H], f32, tag="dzr")
                    rsum = dnp.tile([128, 1], f32, tag="drs")
                    nc.scalar.activation(zr[:], zb[:], mybir.ActivationFunctionType.Relu,
                                         accum_out=rsum[:])
                    mean = dnp.tile([128, 1], f32, tag="dmn")
                    nc.scalar.mul(mean[:], rsum[:], 1.0 / H)
                    zc = dnp.tile([128, H], f32, tag="dzc")
                    nc.vector.tensor_scalar(zc[:], zr[:], mean[:], None,
                                            mybir.AluOpType.subtract)
                    sq = dnp.tile([128, H], f32, tag="dsq")
                    ssq = dnp.tile([128, 1], f32, tag="dsq1")
                    nc.scalar.activation(sq[:], zc[:], mybir.ActivationFunctionType.Square,
                                         accum_out=ssq[:])
                    std = dnp.tile([128, 1], f32, tag="dsd")
                    nc.scalar.activation(std[:], ssq[:], mybir.ActivationFunctionType.Sqrt,
                                         bias=eps_t[:], scale=1.0 / H)
                    inv = dnp.tile([128, 1], f32, tag="din")
                    nc.vector.reciprocal(inv[:], std[:])
                    t1 = dnp.tile([128, H], f32, tag="dt1")
                    nc.vector.scalar_tensor_tensor(t1[:], zc[:], inv[:], bc_t[:, LNG, :],
                                                   mybir.AluOpType.mult, mybir.AluOpType.mult)
                    hout = dnp.tile([128, H], f32, tag="dho")
                    nc.vector.tensor_tensor(hout[:], t1[:], bc_t[:, LNB, :], mybir.AluOpType.add)
                    out_cb(t, hout)

            # ---------- round 1 ----------
            if STAGE >= 2:
                spmm_round(H0_d, s1_d)

            def r1_out(t, hout):
                nc.sync.dma_start(h1_d[t * 128 : (t + 1) * 128, :], hout[:])

            if STAGE >= 3:
                dense_round(s1_d, w1T_t, B1, r1_out)
                nc.gpsimd.collective_compute(
                    "AllGather", mybir.AluOpType.bypass, replica_groups=RG,
                    ins=[h1_d[0:SHARD, :]], outs=[H1_d[:]],
                )

            # ---------- round 2 + residual + item_id_emb ----------
            if STAGE >= 4:
                spmm_round(H1_d, s2_d)

            def r2_out(t, hout):
                r0 = dnp.tile([128, H], f32, tag="dr0")
                nc.sync.dma_start(r0[:], h0_d[t * 128 : (t + 1) * 128, :])
                ii = dnp.tile([128, H], f32, tag="dii")
                nc.sync.dma_start(ii[:], iid_d[t * 128 : (t + 1) * 128, :])
                e1 = dnp.tile([128, H], f32, tag="de1")
                nc.vector.tensor_tensor(e1[:], hout[:], r0[:], mybir.AluOpType.add)
                e2 = dnp.tile([128, H], f32, tag="de2")
                nc.vector.tensor_tensor(e2[:], e1[:], ii[:], mybir.AluOpType.add)
                nc.sync.dma_start(item_d[t * 128 : (t + 1) * 128, :], e2[:])

            if STAGE >= 4:
                dense_round(s2_d, w2T_t, B2, r2_out)

            # ---------- scoring ----------
            bs_t = scp.tile([128, BMAX // 128], f32)
            nc.sync.dma_start(bs_t[:], bsum_d[:])
            if STAGE >= 5:
                iix_t = scp.tile([128, BMAX // 16], i16)
                nc.sync.dma_start(iix_t[:], iidx_d[:])
                it_t = scp.tile([128, BMAX // 128, H], f32)
                nc.gpsimd.dma_gather(it_t[:], item_d[:], iix_t[:], BMAX, BMAX, H)
                u_t = scp.tile([128, BMAX // 128, H], f32)
                nc.sync.dma_start(u_t[:], urows_d.ap().rearrange("(c p) h -> p c h", p=128))
                pr_t = scp.tile([128, BMAX // 128, H], f32)
                nc.vector.tensor_tensor(pr_t[:], it_t[:], u_t[:], mybir.AluOpType.mult)
                dot_t = scp.tile([128, BMAX // 128], f32)
                nc.vector.tensor_reduce(dot_t[:], pr_t[:], mybir.AxisListType.X,
                                        mybir.AluOpType.add)
                sco_t = scp.tile([128, BMAX // 128], f32)
                nc.vector.tensor_tensor(sco_t[:], dot_t[:], bs_t[:], mybir.AluOpType.add)
                nc.sync.dma_start(scores_d[:], sco_t[:])
            else:
                nc.sync.dma_start(scores_d[:], bs_t[:])

    nc.compile()
    return nc


def kernel(**inputs):
    global _compiled
    nf = np.asarray(inputs["node_features"], np.float32)
    adj_row = np.asarray(inputs["adj_row"], np.int64)
    adj_col = np.asarray(inputs["adj_col"], np.int64)
    adj_vals = np.asarray(inputs["adj_vals"], np.float32)
    user_idx = np.asarray(inputs["user_idx"], np.int64)
    item_idx = np.asarray(inputs["item_idx"], np.int64)
    W_proj = np.asarray(inputs["W_proj"], np.float32)
    b_proj = np.asarray(inputs["b_proj"], np.float32)
    W1 = np.asarray(inputs["W1"], np.float32)
    b1 = np.asarray(inputs["b1"], np.float32)
    W2 = np.asarray(inputs["W2"], np.float32)
    b2 = np.asarray(inputs["b2"], np.float32)
    ln_g = np.asarray(inputs["ln_g"], np.float32)
    ln_b = np.asarray(inputs["ln_b"], np.float32)
    user_emb = np.asarray(inputs["user_emb"], np.float32)
    item_id_emb = np.asarray(inputs["item_id_emb"], np.float32)
    user_bias = np.asarray(inputs["user_bias"], np.float32)
    item_bias = np.asarray(inputs["item_bias"], np.float32)
    global_bias = np.asarray(inputs["global_bias"], np.float32)

    if _compiled is None:
        _compiled = _build()
    nc = _compiled

    bcasts = np.ascontiguousarray(np.stack([
        np.broadcast_to(b_proj, (128, H)),
        np.broadcast_to(b1, (128, H)),
        np.broadcast_to(b2, (128, H)),
        np.broadcast_to(ln_g, (128, H)),
        np.broadcast_to(ln_b, (128, H)),
    ]).transpose(1, 0, 2)).astype(np.float32)
    ident = np.eye(128, dtype=np.float32)

    in_maps = []
    routing = []
    core_of = item_idx // SHARD
    for c in range(NCORES):
        gidx, sidx, vals = _prep_spmm(c, adj_row, adj_col, adj_vals)
        nfT = np.zeros((F_DIM, SHARD_PAD), np.float32)
        nfT[:, :SHARD] = nf[c * SHARD : (c + 1) * SHARD].T
        iid = np.zeros((SHARD_PAD, H), np.float32)
        iid[:SHARD] = item_id_emb[c * SHARD : (c + 1) * SHARD]

        pos = np.nonzero(core_of == c)[0]
        if len(pos) > BMAX:
            raise RuntimeError(f"core {c}: {len(pos)} routed items > {BMAX}")
        il = np.zeros(BMAX, np.int64)
        il[: len(pos)] = item_idx[pos] - c * SHARD
        ur = np.zeros((BMAX, H), np.float32)
        ur[: len(pos)] = user_emb[user_idx[pos]]
        bs = np.zeros(BMAX, np.float32)
        bs[: len(pos)] = (
            user_bias[user_idx[pos], 0] + item_bias[item_idx[pos], 0] + global_bias[0]
        )
        routing.append(pos)
        in_maps.append({
            "nfT": nfT, "wpT": W_proj.T.copy(), "w1T": W1.T.copy(), "w2T": W2.T.copy(),
            "bcasts": bcasts, "iid": iid, "ident": ident,
            "gidx": gidx, "sidx": sidx, "vals": vals,
            "iidx": _wrap16(il), "urows": ur, "bsum": _tokwrap(bs),
        })

    from concourse.bass_utils import run_bass_kernel_spmd
    import time as _time
    _t = _time.time()
    res = run_bass_kernel_spmd(nc, in_maps, list(range(NCORES)))
    global last_run_ns
    last_run_ns = int((_time.time() - _t) * 1e9)

    out = np.zeros(BATCH, np.float32)
    for c in range(NCORES):
        sc = res.results[c]["scores"]
        pos = routing[c]
        t = np.arange(len(pos))
        out[pos] = sc[t % 128, t // 128]
    return out
